# revision 28
# baseline (speedup 1.0000x reference)
"""DPFP multi-head linear attention kernel for 8 Trainium2 NeuronCores.

Sharding: data-parallel over batch (B=2 -> two groups of 4 cores), tensor-
parallel over heads (16 heads -> 4 heads per core), row-sharded o-projection
with a chunked ReduceScatter over each 4-core group, residual+LayerNorm on
the scattered rows.

Math (per batch b, per head n):
  q = h @ Wq, k,v = split(h @ Wkv)
  pq = dpfp(q), pk = dpfp(k)          # dpfp: [relu(x), relu(-x)] then
                                      # concat_i x2 * roll(x2, i), i=1..3
  kvzT = [v | 1]^T-contract-S pk      # [D+1, F]; row D = z = sum_j pk
  numT/den = kvz-contract-F pqT       # [D+1, S] per head
  attn_vecT = numT / (den + EPS/SCALE)  # SCALE cancels between num and den
  out = LN(h + attn_vec @ Wo)

All matmuls run as float32r (fp32 data, tf32-like mantissa, full PE rate).
Set BASS_KERNEL_FP32=1 to force exact fp32 matmuls (4x slower PE).
"""
import contextlib
import os
import sys

sys.path.insert(0, "/opt/trn_rl_repo")

import numpy as np

import concourse.bacc as bacc
import concourse.mybir as mybir
import concourse.tile as tile
from concourse.bass_utils import run_bass_kernel_spmd

AF = mybir.ActivationFunctionType
ALU = mybir.AluOpType
F32 = mybir.dt.float32
BF16 = mybir.dt.bfloat16
F32R = mybir.dt.float32 if os.environ.get("BASS_KERNEL_FP32") else mybir.dt.float32r
# dtype for the dpfp feature map (x2, pk, pq, vext, kvz, pqT): bf16 halves DVE
# mul time and PE transpose time; the num/den ratio cancels the rounding.
FMAP = F32R if os.environ.get("BASS_KERNEL_FMAP_F32") else mybir.dt.bfloat16
QMAP = F32R if os.environ.get("BASS_KERNEL_QMAP_F32") else mybir.dt.bfloat16

S = 2048
B = 2
D = 1024
NH = 16          # total heads
NHC = 4          # heads per core
DH = 64
FD = 384         # dpfp feature dim per head = 2*DH*3
EPS_EFF = 8e-5   # EPS / SCALE = 1e-5 * sqrt(64)
DP = DH + 4      # padded [v|1] width for even PE-transpose dims
N_CORES = 8
GRP = 4          # cores per reduce-scatter group
SCH = S // 128   # 16 s-chunks of 128
SG = 4           # s-groups of 512
KC = D // 128    # 8 contraction chunks


def _emit_proj(nc, tc, ev, io, const, sb_hT, sb_hld, sb_qkv, ps):
    """h -> hT (transposed), then q/k/v projections in natural layout."""
    (h_in, _hres_in, _out_p, _rs_in, _rs_out, _r_dram) = io
    (wq_sb, wkv_sb, _wo_sb, _gb_sb, identr, _onesf, _epst, _epse, _onesrow, _identb, _identq) = const

    hT = sb_hT.tile([128, KC, S], F32R, tag="hT")
    q_nat = sb_qkv.tile([128, SCH, NHC * DH], QMAP, tag="q_nat")
    kv_sb = sb_qkv.tile([128, SCH, 2 * NHC * DH], FMAP, tag="kv_sb")
    # Interleave per s-chunk: transpose chunk i, then immediately project it,
    # so proj matmuls overlap later chunks' DMA + transposes.
    for i in range(SCH):
        h_tile = sb_hld.tile([128, D], F32R, tag="hld", bufs=3)
        nc.gpsimd.dma_start(out=h_tile[:], in_=h_in[i * 128:(i + 1) * 128, :])
        for cpair in range(KC // 4):
            pt = ps.tile([128, 4, 128], F32R, tag="ps")
            for j in range(4):
                c = cpair * 4 + j
                nc.tensor.transpose(
                    pt[:, j, :], h_tile[:, c * 128:(c + 1) * 128], identr[:])
            ev()(
                hT[:, cpair * 4:cpair * 4 + 4, i * 128:(i + 1) * 128], pt[:])
        pkv = ps.tile([128, 512], F32, tag="ps")
        pq = ps.tile([128, 256], F32, tag="ps2", bufs=2)
        for c in range(KC):
            lhsT = hT[:, c, i * 128:(i + 1) * 128]
            nc.tensor.matmul(pkv[:], lhsT,
                             wkv_sb[:, c, :], start=(c == 0), stop=(c == KC - 1))
            nc.tensor.matmul(pq[:], lhsT,
                             wq_sb[:, c, :], start=(c == 0), stop=(c == KC - 1))
        ev()(kv_sb[:, i, :], pkv[:])
        ev()(q_nat[:, i, :], pq[:])
    return q_nat, kv_sb


def _dpfp(nc, sb_head, x2_src, tag_x2, tag_out):
    """relu halves into x2, then rolled multiplies -> [128, SCH, FD] f32r."""
    x2 = sb_head.tile([128, SCH, 128], FMAP, tag=tag_x2, name=tag_x2)
    nc.scalar.activation(x2[:, :, 0:DH], x2_src, AF.Relu)
    nc.scalar.activation(x2[:, :, DH:128], x2_src, AF.Relu, scale=-1.0)
    p = sb_head.tile([128, SCH, FD], FMAP, tag=tag_out, name=tag_out)
    for r in (1, 2, 3):
        base = (r - 1) * 128
        eng = nc.gpsimd if r == 2 else nc.vector
        eng.tensor_tensor(out=p[:, :, base + r:base + 128],
                          in0=x2[:, :, r:128], in1=x2[:, :, 0:128 - r],
                          op=ALU.mult)
        nc.vector.tensor_tensor(out=p[:, :, base:base + r],
                                in0=x2[:, :, 0:r], in1=x2[:, :, 128 - r:128],
                                op=ALU.mult)
    return p


def _emit_attn_head_prep(nc, tc, ev, const, q_nat, kv_sb, kvz_all, sb_head, ps):
    """Phase A: per head, dpfp(k) -> pk -> kvzT -> kvz (F-major)."""
    (_wq_sb, _wkv_sb, _wo_sb, _gb_sb, identr, onesf, _epst, epse, onesrow, identb, identq) = const
    for n in range(NHC):
        pk = _dpfp(nc, sb_head, kv_sb[:, :, n * DH:(n + 1) * DH], "x2", "pk")
        vext = sb_head.tile([128, SCH, DP], FMAP, tag="vext")
        nc.scalar.copy(vext[:, :, 0:DH],
                       kv_sb[:, :, NHC * DH + n * DH:NHC * DH + (n + 1) * DH])
        nc.scalar.copy(vext[:, :, DH:DP], onesf[:])
        pkvz = ps.tile([DP, FD], F32, tag="ps")
        for i in range(SCH):
            nc.tensor.matmul(pkvz[:], vext[:, i, :], pk[:, i, :],
                             start=(i == 0), stop=(i == SCH - 1))
        kvzT = sb_head.tile([DP, FD], FMAP, tag="kvzT")
        nc.scalar.copy(kvzT[:], pkvz[:])
        for r in range(3):
            ptr = ps.tile([128, DP], FMAP, tag="ps")
            nc.tensor.transpose(ptr[:], kvzT[:, r * 128:(r + 1) * 128],
                                identb[0:DP, 0:DP])
            nc.vector.tensor_copy(kvz_all[:, n, r, :], ptr[:])


def _emit_pq_head(nc, ev, const, n, q_nat, pqT_all, sb_head, ps):
    """dpfp(q) for head n over the FULL sequence, transposed into pqT_all."""
    (_wq, _wkv, _wo, _gb, _identr, _onesf, _epst, _epse, _onesrow, _identb, identq) = const
    x2q = sb_head.tile([128, SCH, 128], QMAP, tag="x2q", bufs=2, name="x2q")
    src = q_nat[:, :, n * DH:(n + 1) * DH]
    nc.scalar.activation(x2q[:, :, 0:DH], src, AF.Relu)
    nc.scalar.activation(x2q[:, :, DH:128], src, AF.Relu, scale=-1.0)
    pq = sb_head.tile([128, SCH, FD], QMAP, tag="pq_g", bufs=1, name="pq")
    for r in (1, 2, 3):
        base = (r - 1) * 128
        eng = nc.gpsimd if r == 2 else nc.vector
        eng.tensor_tensor(out=pq[:, :, base + r:base + 128],
                          in0=x2q[:, :, r:128], in1=x2q[:, :, 0:128 - r],
                          op=ALU.mult)
        nc.vector.tensor_tensor(out=pq[:, :, base:base + r],
                                in0=x2q[:, :, 0:r], in1=x2q[:, :, 128 - r:128],
                                op=ALU.mult)
    for r in range(3):
        for q4 in range(SCH // 4):
            ptt = ps.tile([128, 4, 128], QMAP, tag="ps")
            for j in range(4):
                i = q4 * 4 + j
                nc.tensor.transpose(ptt[:, j, :],
                                    pq[:, i, r * 128:(r + 1) * 128], identq[:])
            ev()(pqT_all[:, n, r, q4 * 512:(q4 + 1) * 512], ptt[:])


def _emit_group(nc, tc, ev, io, const, g, pqT_all, kvz_all, avT_g, sb_head, sb_ln, ps):
    """Phase B for s-group g: per head num/divide from precomputed pqT, then
    o-projection and the rs_in staging writes."""
    (_h_in, hres_in, out_p, rs_in, rs_out, _r_dram) = io
    (_wq_sb, _wkv_sb, wo_sb, gb_sb, identr, onesf, epst, epse, onesrow, identb, identq) = const

    for n in range(NHC):
        pnd = ps.tile([DH + 1, 512], F32, tag="ps")
        for r in range(3):
            nc.tensor.matmul(pnd[:], kvz_all[:, n, r, 0:DH + 1],
                             pqT_all[:, n, r, g * 512:(g + 1) * 512],
                             start=(r == 0), stop=(r == 2))
        den = sb_head.tile([1, 512], F32R, tag="den", bufs=4)
        nc.scalar.activation(den[:], pnd[DH:DH + 1, :], AF.Identity,
                             bias=epse[0:1, :])
        pden = ps.tile([64, 512], F32, tag="ps")
        nc.tensor.matmul(pden[:], onesrow[:], den[:], start=True, stop=True)
        r_bc = sb_head.tile([64, 512], F32, tag="r_bc", bufs=4)
        nc.vector.reciprocal(r_bc[:], pden[:])
        nc.vector.tensor_tensor(
            out=avT_g[(n % 2) * 64:(n % 2) * 64 + 64, n // 2, :],
            in0=pnd[0:DH, :], in1=r_bc[:], op=ALU.mult)

    for il in range(4):
        for dg in range(2):
            po = ps.tile([128, 512], F32, tag="ps")
            for m in range(2):
                nc.tensor.matmul(po[:], avT_g[:, m, il * 128:(il + 1) * 128],
                                 wo_sb[:, m, dg * 512:(dg + 1) * 512],
                                 start=(m == 0), stop=(m == 1))
            o_t = sb_ln.tile([128, 512], BF16, tag="o_t", bufs=3)
            ev()(o_t[:], po[:])
            p, gg = (0, g) if g < 2 else (g - 1, 0)
            nc.sync.dma_start(
                out=rs_in[p][il, gg, :, dg * 512:(dg + 1) * 512],
                in_=o_t[:])


def _emit_rs(nc, io, p):
    """One ReduceScatter per PAIR of s-groups: the 15us fixed collective
    overhead is paid twice instead of four times, and the payload is bf16."""
    (_h_in, _hres_in, _out_p, rs_in, rs_out, _r_dram) = io
    if os.environ.get("NO_COLL"):
        nc.sync.dma_start(out=rs_out[p][:], in_=rs_in[p][0, :, :, :])
    else:
        nc.gpsimd.collective_compute(
            "ReduceScatter", ALU.add,
            replica_groups=[[0, 1, 2, 3], [4, 5, 6, 7]],
            ins=[rs_in[p][:].opt()],
            outs=[rs_out[p][:].opt()],
        )


def _emit_ln(nc, tc, ev, io, const, g, sb_ln):
    """Residual + LayerNorm for group g; emitted lagged so the engines'
    in-order streams don't head-of-line block on group g's ReduceScatter."""
    (_h_in, hres_in, out_p, _rs_in, rs_out, _r_dram) = io
    (_wq_sb, _wkv_sb, _wo_sb, gb_sb, _identr, _onesf, epst, _epse, _onesrow, _identb, _identq) = const
    xb = sb_ln.tile([128, D], BF16, tag="xb", bufs=2)
    p, gg = (0, g) if g < 2 else (g - 1, 0)
    nc.sync.dma_start(out=xb[:], in_=rs_out[p][gg, :, :])
    hres = sb_ln.tile([128, D], F32, tag="hres", bufs=1)
    nc.sync.dma_start(out=hres[:], in_=hres_in[g])
    x = sb_ln.tile([128, D], F32, tag="x", bufs=2)
    nc.gpsimd.tensor_tensor(out=x[:], in0=xb[:], in1=hres[:], op=ALU.add)
    stats = sb_ln.tile([128, 2, 6], F32, tag="stats", bufs=2)
    xg = x[:].rearrange("p (s f) -> p s f", s=2)
    for si in range(2):
        nc.vector.bn_stats(out=stats[:, si, :], in_=xg[:, si, :])
    mv = sb_ln.tile([128, 2], F32, tag="mv", bufs=2)
    nc.vector.bn_aggr(out=mv[:], in_=stats[:])
    rstd = sb_ln.tile([128, 1], F32, tag="rstd", bufs=2)
    nc.scalar.activation(rstd[:], mv[:, 1:2], AF.Sqrt, bias=epst[:])
    nc.vector.reciprocal(rstd[:], rstd[:])
    t1 = sb_ln.tile([128, D], F32, tag="t1", bufs=1)
    nc.vector.scalar_tensor_tensor(out=t1[:], in0=x[:], scalar=mv[:, 0:1],
                                   in1=gb_sb[:, 0, :], op0=ALU.subtract,
                                   op1=ALU.mult)
    o_f = sb_ln.tile([128, D], F32, tag="o_f", bufs=1)
    nc.gpsimd.tensor_scalar(out=o_f[:], in0=t1[:], scalar1=rstd[:],
                            scalar2=None, op0=ALU.mult)
    nc.gpsimd.tensor_tensor(out=o_f[:], in0=o_f[:], in1=gb_sb[:, 1, :],
                            op=ALU.add)
    nc.sync.dma_start(out=out_p[g], in_=o_f[:])


def build(reps=1):
    nc = bacc.Bacc("TRN2", target_bir_lowering=False, debug=False)

    h_in = nc.dram_tensor("h", [S, D], F32, kind="ExternalInput")
    hres_in = nc.dram_tensor("hres", [SG, 128, D], F32, kind="ExternalInput")
    wq_in = nc.dram_tensor("wq", [D, NHC * DH], F32, kind="ExternalInput")
    wkv_in = nc.dram_tensor("wkv", [D, 2 * NHC * DH], F32, kind="ExternalInput")
    wo_in = nc.dram_tensor("wo", [NHC * DH, D], F32, kind="ExternalInput")
    gamma_in = nc.dram_tensor("gamma", [D], F32, kind="ExternalInput")
    beta_in = nc.dram_tensor("beta", [D], F32, kind="ExternalInput")
    out_p = nc.dram_tensor("out", [SG, 128, D], F32, kind="ExternalOutput")

    # Split 2+1+1: ReduceScatter for groups {0,1} right after group 1, then
    # one per remaining group — each overlaps the next group's compute and
    # the last exposed collective is small.
    rs_in = [nc.dram_tensor("rs_bounce_in0", [GRP, 2, 128, D], BF16),
             nc.dram_tensor("rs_bounce_in1", [GRP, 1, 128, D], BF16),
             nc.dram_tensor("rs_bounce_in2", [GRP, 1, 128, D], BF16)]
    rs_out = [nc.dram_tensor("rs_bounce_out0", [2, 128, D], BF16),
              nc.dram_tensor("rs_bounce_out1", [1, 128, D], BF16),
              nc.dram_tensor("rs_bounce_out2", [1, 128, D], BF16)]
    io = (h_in, hres_in, out_p, rs_in, rs_out, None)

    ev_state = [0]

    with tile.TileContext(nc) as tc:
        def ev():
            ev_state[0] += 1
            if ev_state[0] % 2:
                return nc.vector.tensor_copy
            return nc.scalar.copy

        with contextlib.ExitStack() as ctx:
            sb_c = ctx.enter_context(tc.tile_pool(name="const", bufs=1))
            ps = ctx.enter_context(tc.tile_pool(name="psum", bufs=6, space="PSUM"))
            sb_qkv = ctx.enter_context(tc.tile_pool(name="qkv", bufs=1))
            sb_attn = ctx.enter_context(tc.tile_pool(name="attn", bufs=1))

            wq_sb = sb_c.tile([128, KC, NHC * DH], F32R)
            nc.gpsimd.dma_start(out=wq_sb[:],
                                in_=wq_in[:].rearrange("(c p) m -> p c m", p=128))
            wkv_sb = sb_c.tile([128, KC, 2 * NHC * DH], F32R)
            nc.gpsimd.dma_start(out=wkv_sb[:],
                                in_=wkv_in[:].rearrange("(c p) m -> p c m", p=128))
            wo_sb = sb_c.tile([128, 2, D], F32R)
            nc.gpsimd.dma_start(out=wo_sb[:],
                                in_=wo_in[:].rearrange("(c p) m -> p c m", p=128))
            gb_sb = sb_c.tile([128, 2, D], F32)
            nc.sync.dma_start(out=gb_sb[:, 0, :],
                              in_=gamma_in[:].partition_broadcast(128))
            nc.sync.dma_start(out=gb_sb[:, 1, :],
                              in_=beta_in[:].partition_broadcast(128))
            ident = sb_c.tile([128, 128], F32)
            nc.gpsimd.memset(ident[:], 0.0)
            nc.gpsimd.affine_select(out=ident[:], in_=ident[:],
                                    compare_op=ALU.not_equal, fill=1.0, base=0,
                                    pattern=[[-1, 128]], channel_multiplier=1)
            identr = sb_c.tile([128, 128], F32R)
            nc.scalar.copy(identr[:], ident[:])
            identb = sb_c.tile([128, 128], FMAP)
            nc.scalar.copy(identb[:], ident[:])
            identq = sb_c.tile([128, 128], QMAP)
            nc.scalar.copy(identq[:], ident[:])
            onesf = sb_c.tile([128, SCH, DP - DH], F32)
            nc.vector.memset(onesf[:], 1.0)
            epst = sb_c.tile([128, 1], F32)
            nc.vector.memset(epst[:], 1e-5)
            epse = sb_c.tile([128, 1], F32)
            nc.vector.memset(epse[:], EPS_EFF)
            onesrow_f = sb_c.tile([1, 64], F32)
            nc.vector.memset(onesrow_f[:], 1.0)
            onesrow = sb_c.tile([1, 64], F32R)
            nc.scalar.copy(onesrow[:], onesrow_f[:])
            const = (wq_sb, wkv_sb, wo_sb, gb_sb, identr, onesf, epst, epse, onesrow, identb, identq)

            phases = os.environ.get("PHASES", "full")
            for _rep in range(reps):
                with tc.tile_pool(name="hT", bufs=1) as sb_hT, \
                     tc.tile_pool(name="hld", bufs=1) as sb_hld:
                    q_nat, kv_sb = _emit_proj(nc, tc, ev, io, const,
                                              sb_hT, sb_hld, sb_qkv, ps)
                if phases == "proj":
                    for g in range(SG):
                        nc.sync.dma_start(out=out_p[g][:, 0:512],
                                          in_=kv_sb[:, g, :])
                    continue
                with tc.tile_pool(name="head", bufs=1) as sb_head, \
                     tc.tile_pool(name="ln", bufs=1) as sb_ln:
                    kvz_all = sb_head.tile([128, NHC, 3, DP], QMAP, tag="kvz_all")
                    _emit_attn_head_prep(nc, tc, ev, const, q_nat, kv_sb,
                                         kvz_all, sb_head, ps)
                    pqT_all = sb_attn.tile([128, NHC, 3, S], QMAP, tag="pqT_all")
                    for n in range(NHC):
                        _emit_pq_head(nc, ev, const, n, q_nat, pqT_all,
                                      sb_head, ps)
                    for g in range(SG):
                        avT_g = sb_attn.tile([128, NHC // 2, 512], F32R,
                                             tag="avT", bufs=2)
                        _emit_group(nc, tc, ev, io, const, g, pqT_all, kvz_all,
                                    avT_g, sb_head, sb_ln, ps)
                        if g >= 1:
                            _emit_rs(nc, io, g - 1)
                        if g >= 2:
                            _emit_ln(nc, tc, ev, io, const, g - 2, sb_ln)
                    _emit_ln(nc, tc, ev, io, const, SG - 2, sb_ln)
                    _emit_ln(nc, tc, ev, io, const, SG - 1, sb_ln)
    nc.compile()
    return nc


_NC_CACHE = {}


def _get_nc(reps=1):
    if reps not in _NC_CACHE:
        _NC_CACHE[reps] = build(reps)
    return _NC_CACHE[reps]


def make_in_maps(h, Wq, Wkv, Wo, ln_gamma, ln_beta):
    h = np.asarray(h, dtype=np.float32)
    Wq = np.asarray(Wq, dtype=np.float32)
    Wkv = np.asarray(Wkv, dtype=np.float32)
    Wo = np.asarray(Wo, dtype=np.float32)
    g = np.ascontiguousarray(np.asarray(ln_gamma, dtype=np.float32))
    be = np.ascontiguousarray(np.asarray(ln_beta, dtype=np.float32))
    in_maps = []
    for c in range(N_CORES):
        b, hg, r = c // GRP, c % GRP, c % GRP
        h_b = np.ascontiguousarray(h[:, b, :])
        hres = np.stack([h_b[gi * 512 + r * 128: gi * 512 + (r + 1) * 128]
                         for gi in range(SG)])
        cs = hg * NHC * DH
        in_maps.append({
            "h": h_b,
            "hres": np.ascontiguousarray(hres),
            "wq": np.ascontiguousarray(Wq[:, cs:cs + 256]),
            "wkv": np.ascontiguousarray(
                np.concatenate([Wkv[:, cs:cs + 256],
                                Wkv[:, NH * DH + cs:NH * DH + cs + 256]],
                               axis=1)),
            "wo": np.ascontiguousarray(Wo[cs:cs + 256, :]),
            "gamma": g,
            "beta": be,
        })
    return in_maps


def assemble(results):
    out = np.empty((S, B, D), dtype=np.float32)
    for c in range(N_CORES):
        b, r = c // GRP, c % GRP
        o = results[c]["out"]
        for gi in range(SG):
            out[gi * 512 + r * 128: gi * 512 + (r + 1) * 128, b, :] = o[gi]
    return out


def run(in_maps, reps=1):
    nc = _get_nc(reps)
    return run_bass_kernel_spmd(nc, in_maps, list(range(N_CORES)))


def kernel(h, Wq, Wkv, Wo, ln_gamma, ln_beta):
    in_maps = make_in_maps(h, Wq, Wkv, Wo, ln_gamma, ln_beta)
    # The first execution right after a fresh compile occasionally hits a
    # transient "mesh desynced" collective error while the NEFF is still
    # loading on some cores; a retry on the (now cached) NEFF succeeds.
    last = None
    for _ in range(3):
        try:
            res = run(in_maps, reps=1)
            return assemble(res.results)
        except Exception as e:  # noqa: BLE001
            last = e
    raise last

